# revision 1
# baseline (speedup 1.0000x reference)
"""Trainium2 Bass kernel for nn_NeuralSurface (8-layer MLP SDF with harmonic
embedding + skip concat), data-parallel over 8 NeuronCores.

Layout strategy: activations kept transposed in SBUF ([features, points]),
weights stationary fp16, PE matmuls K/M-chunked to 128. Harmonic sin/cos via
ScalarE Sin LUT after DVE range reduction to [-pi, pi] (magic-number
round-to-nearest). ReLU+bias split between ScalarE (activation Relu w/ bias)
and VectorE (tensor_scalar add+max) reading PSUM. n-tiles processed in pairs
so the PE always has independent matmul work while ReLUs complete.
"""

import numpy as np

import concourse.bacc as bacc
import concourse.mybir as mybir
import concourse.tile as tile
from concourse.bass_utils import run_bass_kernel_spmd

AF = mybir.ActivationFunctionType
ALU = mybir.AluOpType
F32 = mybir.dt.float32
F16 = mybir.dt.float16

N_CORES = 8
N = 262144
NPC = N // N_CORES  # 32768 points per core
NT = 512  # points per n-tile (PSUM bank / fp32 moving-operand limit)
PAIRS = NPC // (2 * NT)  # 32
H = 256
E = 39
NHARM = 6
TWO_PI = float(2.0 * np.pi)
MAGIC = float(1.5 * 2.0**23)  # round-to-nearest via (x + M) - M

# ReLU engine split: half 0 -> ACT, half 1 -> DVE (even split; each PSUM
# pair drains through two engines in parallel).
DVE_RELU = {(li, 1): True for li in range(8)}

_CACHED = {}


def _build():
    nc = bacc.Bacc("TRN2")

    rep6 = nc.dram_tensor("rep6", [128, NPC], F32, kind="ExternalInput").ap()
    ptsh = nc.dram_tensor("ptsh", [3, NPC], F16, kind="ExternalInput").ap()
    w0h = nc.dram_tensor("w0h", [128, H], F16, kind="ExternalInput").ap()
    wkh = {
        i: nc.dram_tensor(f"w{i}h", [H, H], F16, kind="ExternalInput").ap()
        for i in (1, 2, 3, 5, 6, 7)
    }
    w4eh = nc.dram_tensor("w4eh", [128, H], F16, kind="ExternalInput").ap()
    w4ah = nc.dram_tensor("w4ah", [128, H], F16, kind="ExternalInput").ap()
    w4bh = nc.dram_tensor("w4bh", [128, H], F16, kind="ExternalInput").ap()
    wsdfh = nc.dram_tensor("wsdfh", [H, 1], F16, kind="ExternalInput").ap()
    bmat = nc.dram_tensor("bmat", [128, 16], F32, kind="ExternalInput").ap()
    bsdf1 = nc.dram_tensor("bsdf1", [128, 1], F32, kind="ExternalInput").ap()
    # 2-D output (1-D ExternalOutput tensors fail NEFF load under bass2jax)
    out_o = nc.dram_tensor("out_o", [NPC // NT, NT], F32, kind="ExternalOutput").ap()

    with tile.TileContext(nc) as tc:
        with (
            tc.tile_pool(name="wp", bufs=1) as wp,
            tc.tile_pool(name="ep", bufs=4) as ep,
            tc.tile_pool(name="hp", bufs=4) as hp,
            tc.tile_pool(name="op", bufs=4) as op_,
            tc.tile_pool(name="pp", bufs=6, space="PSUM") as pp,
            tc.tile_pool(name="pf", bufs=1, space="PSUM") as pf,
        ):
            # ---- one-time weight / const loads ----
            w0s = wp.tile_from(w0h, name="w0s")  # [39, 256]
            wks = {
                i: (
                    wp.tile_from(wkh[i][0:128, :], name=f"wks{i}a"),
                    wp.tile_from(wkh[i][128:256, :], name=f"wks{i}b"),
                )
                for i in (1, 2, 3, 5, 6, 7)
            }
            w4es = wp.tile_from(w4eh, name="w4es")  # [128, 256] K-padded
            w4as = wp.tile_from(w4ah, name="w4as")  # [128, 256]
            w4bs = wp.tile_from(w4bh, name="w4bs")
            wsdf_a = wp.tile_from(wsdfh[0:128, :], name="wsdf_a")  # [128, 1]
            wsdf_b = wp.tile_from(wsdfh[128:256, :], name="wsdf_b")
            bms = wp.tile_from(bmat, name="bms")  # [128, 16]
            bsdfs = wp.tile_from(bsdf1, name="bsdfs")  # [1, 1]
            zcol = wp.tile([128, 1], F32, name="zcol")
            nc.vector.memset(zcol, 0.0)

            def wchunk(i, k, m):
                # lhsT [128, 128] slice: layer i, K-chunk k, M-half m
                return wks[i][k][:, bass_ts(m, 128)]

            for p in range(PAIRS):
                s = p * 2 * NT  # start point index of the pair (A at s, B at s+NT)
                W = 2 * NT  # pair-wide free size

                # ---- embedding (pair-wide, [128, 1024] ops) ----
                # rep6 rows carry t0 = x*2^j/(2pi) + phase (host-precomputed
                # exact scaling); rows 36:128 are zero -> Sin gives 0, so emb
                # is K-padded to 128 for free (full-K weight loads on PE).
                t0 = ep.tile([128, W], F32, tag="t0")
                nc.sync.dma_start(out=t0, in_=rep6[:, s:s + W])
                rr = ep.tile([128, W], F32, tag="rr")
                nc.vector.tensor_scalar(rr, t0, MAGIC, MAGIC, op0=ALU.add, op1=ALU.subtract)
                ys = ep.tile([128, W], F32, tag="ys")
                nc.vector.tensor_tensor(out=ys, in0=t0, in1=rr, op=ALU.subtract)

                emb = ep.tile([128, W], F16, tag="emb")
                nc.scalar.activation(emb, ys, AF.Sin, bias=zcol, scale=TWO_PI)
                nc.sync.dma_start(out=emb[36:39, :], in_=ptsh[:, s:s + W])

                # ---- MLP layers ----
                # h tile layout: [128, 4*NT]: A-half0, A-half1, B-half0, B-half1
                h_prev = None
                h3 = None
                for li in range(8):
                    h = hp.tile([128, 4 * NT], F16, tag="h")
                    # chunks: list of (weight tile [128,256], rhs per half_x)
                    if li == 0:
                        chunks = [(w0s, lambda hx: emb[:, bass_ts(hx, NT)])]
                    elif li == 4:
                        chunks = [
                            (w4es, lambda hx: emb[:, bass_ts(hx, NT)]),
                            (w4as, lambda hx, hp3=h3: hp3[:, bass_ts(2 * hx, NT)]),
                            (w4bs, lambda hx, hp3=h3: hp3[:, bass_ts(2 * hx + 1, NT)]),
                        ]
                    else:
                        chunks = [
                            (wks[li][0], lambda hx, hp_=h_prev: hp_[:, bass_ts(2 * hx, NT)]),
                            (wks[li][1], lambda hx, hp_=h_prev: hp_[:, bass_ts(2 * hx + 1, NT)]),
                        ]
                    ps = {(hx, m): pp.tile([128, NT], F32, tag="mm", name="psmm")
                          for hx in range(2) for m in range(2)}
                    last = len(chunks) - 1
                    for hx in range(2):
                        for m in range(2):
                            for ci, (wt, rhs) in enumerate(chunks):
                                nc.tensor.matmul(
                                    ps[(hx, m)], wt[:, bass_ts(m, 128)], rhs(hx),
                                    start=(ci == 0), stop=(ci == last),
                                )
                    # ReLU + bias -> h
                    for half_x in range(2):
                        for m in range(2):
                            dst = h[:, bass_ts(2 * half_x + m, NT)]
                            bias_ap = bms[:, li * 2 + m:li * 2 + m + 1]
                            if DVE_RELU.get((li, m), False):
                                nc.vector.tensor_scalar(
                                    dst, ps[(half_x, m)], bias_ap, 0.0,
                                    op0=ALU.add, op1=ALU.max,
                                )
                            else:
                                nc.scalar.activation(
                                    dst, ps[(half_x, m)], AF.Relu, bias=bias_ap,
                                )
                    if li == 3:
                        h3 = h
                    h_prev = h

                # ---- final SDF layer (M=1), col-group packed: A at array
                # col 0, B at array col 32 -> the two tiles' matmuls overlap
                # on the PE. Separate PSUM banks (same-bank dual accumulation
                # groups + DVE read crashed the exec unit).
                psfa = pf.tile([1, NT], F32, tag="finA")
                psfb_t = pf.tile([33, NT], F32, tag="finB")
                psfb = psfb_t[32:33, :]
                nc.tensor.matmul(
                    psfa, wsdf_a, h_prev[:, bass_ts(0, NT)],
                    start=True, stop=False, tile_position=(0, 0),
                    skip_group_check=True,
                )
                nc.tensor.matmul(
                    psfb, wsdf_a, h_prev[:, bass_ts(2, NT)],
                    start=True, stop=False, tile_position=(0, 32),
                    skip_group_check=True,
                )
                nc.tensor.matmul(
                    psfa, wsdf_b, h_prev[:, bass_ts(1, NT)],
                    start=False, stop=True, tile_position=(0, 0),
                    skip_group_check=True,
                )
                nc.tensor.matmul(
                    psfb, wsdf_b, h_prev[:, bass_ts(3, NT)],
                    start=False, stop=True, tile_position=(0, 32),
                    skip_group_check=True,
                )
                oa = op_.tile([1, NT], F32, tag="oa")
                nc.scalar.activation(oa, psfa, AF.Identity, bias=bsdfs[0:1, 0:1])
                ob = op_.tile([1, NT], F32, tag="ob")
                nc.scalar.activation(ob, psfb, AF.Identity, bias=bsdfs[0:1, 0:1])
                nc.sync.dma_start(out=out_o[2 * p:2 * p + 1, :], in_=oa)
                nc.sync.dma_start(out=out_o[2 * p + 1:2 * p + 2, :], in_=ob)
    nc.compile()
    return nc


def bass_ts(i, size):
    return slice(i * size, (i + 1) * size)


def _prep_maps(points, ws, bs, wsdf, bsdf):
    pts = np.ascontiguousarray(points, dtype=np.float32).reshape(N, 3)
    freqs = (2.0 ** np.arange(NHARM)).astype(np.float32)
    fcol18 = (np.repeat(freqs[None, :], 3, axis=0).reshape(18, 1) / TWO_PI).astype(
        np.float32
    )

    bmat = np.zeros((128, 16), dtype=np.float32)
    for i in range(8):
        for m in range(2):
            bmat[:, i * 2 + m] = bs[i][m * 128:(m + 1) * 128]

    w0p = np.zeros((128, H), dtype=np.float16)
    w0p[0:E, :] = ws[0].astype(np.float16)
    w4ep = np.zeros((128, H), dtype=np.float16)
    w4ep[0:E, :] = ws[4][0:E, :].astype(np.float16)
    common = {
        "w0h": w0p,
        "w4eh": w4ep,
        "w4ah": ws[4][E:E + 128, :].astype(np.float16),
        "w4bh": ws[4][E + 128:E + 256, :].astype(np.float16),
        "wsdfh": wsdf.astype(np.float16),
        "bmat": bmat,
        "bsdf1": np.full((128, 1), float(np.ravel(bsdf)[0]), dtype=np.float32),
    }
    for i in (1, 2, 3, 5, 6, 7):
        common[f"w{i}h"] = ws[i].astype(np.float16)

    in_maps = []
    for c in range(N_CORES):
        sl = pts[c * NPC:(c + 1) * NPC]  # [NPC, 3]
        ptsT = np.ascontiguousarray(sl.T)  # [3, NPC]
        rep3 = np.repeat(ptsT, NHARM, axis=0)  # [18, NPC]
        t18 = rep3 * fcol18  # x * 2^j / (2pi), exact fp32 scaling
        rep6 = np.zeros((128, NPC), dtype=np.float32)
        rep6[0:18], rep6[18:36] = t18, t18 + np.float32(0.25)
        m = dict(common)
        m["rep6"] = rep6
        m["ptsh"] = ptsT.astype(np.float16)
        in_maps.append(m)
    return in_maps


def kernel(
    points, w0, b0, w1, b1, w2, b2, w3, b3, w4, b4, w5, b5, w6, b6, w7, b7,
    wsdf, bsdf,
):
    ws = [np.asarray(w, dtype=np.float32) for w in (w0, w1, w2, w3, w4, w5, w6, w7)]
    bs = [np.asarray(b, dtype=np.float32) for b in (b0, b1, b2, b3, b4, b5, b6, b7)]
    in_maps = _prep_maps(
        np.asarray(points), ws, bs,
        np.asarray(wsdf, dtype=np.float32), np.asarray(bsdf, dtype=np.float32),
    )

    if "nc" not in _CACHED:
        _CACHED["nc"] = _build()
    nc = _CACHED["nc"]

    res = run_bass_kernel_spmd(nc, in_maps, core_ids=list(range(N_CORES)))
    out = np.concatenate(
        [res.results[c]["out_o"] for c in range(N_CORES)], axis=0
    ).reshape(N, 1).astype(np.float32)
    return out

